# revision 11
# baseline (speedup 1.0000x reference)
"""Trainium2 Bass kernel for nn_DAttentionMM (deformable attention, multi-modal).

Strategy: data-parallel over batch B=8 across 8 NeuronCores. Each core runs the
full per-batch pipeline:
  conv3x3(+folded BN)+GELU -> q proj -> offset branch (dwconv/LN/GELU/pw) ->
  bilinear sampling of x, y, h -> sw mixing -> k/v proj -> 8-head attention
  (attnT layout, broadcast-sums AV normalization) -> output proj.

v2: full-bf16 data path (inputs, conv weights, activations, sampling tables,
attention) with f32 PSUM accumulation and f32 offset/position math.  Weights
are bundled into two DRAM params (one bf16, one f32) to cut DMA count; the
hT transposes land in a shared PSUM tile and ship to DRAM as one DMA per
(row-block, channel-half); x/y strip gathers + bilinear run early, hidden
under the conv.
"""
import sys

sys.path.insert(0, '/opt/trn_rl_repo')

import numpy as np
import ml_dtypes

B, C, H, W = 8, 256, 64, 64
NH, HC = 8, 32
Hk = Wk = 8
NS = 64
SCALE = float(HC) ** -0.5
EPS = 1e-5
HW = H * W
PADR = 72          # padded rows/cols for the stride-8 9x9 dwconv (+4 each side)
NROW = PADR * PADR  # 5184

_CACHE = {}

# wbun (bf16) column offsets
_W_PQW, _W_M1W, _W_PKW, _W_PVW, _W_POW, _W_SW2 = 0, 512, 1024, 1536, 2048, 2560
_W_TOT = 2564
# bbun (f32) column offsets
_B_FB, _B_PQB, _B_C1B, _B_SIGB = 0, 2, 4, 6
_B_DWBC, _B_LNG, _B_LNB = 8, 12, 16
_B_PKB, _B_PVB, _B_POB = 20, 22, 24
_B_PWX, _B_PWY = 26, 30
_B_TOT = 34


def _build_program():
    import concourse.bass as bass
    import concourse.tile as tile
    from concourse import bacc, mybir
    from concourse.masks import make_identity

    f32 = mybir.dt.float32
    f32r = mybir.dt.float32r
    bf16 = mybir.dt.bfloat16
    i32 = mybir.dt.int32
    AF = mybir.ActivationFunctionType
    ALU = mybir.AluOpType
    ts = bass.ts

    nc = bacc.Bacc("TRN2", target_bir_lowering=False, debug=False)

    dp = lambda name, shape, dt: nc.declare_dram_parameter(name, list(shape), dt, isOutput=False)
    xc = dp("xc", (C, H, W), bf16)
    yc = dp("yc", (C, H, W), bf16)
    xf = dp("xf", (C, H, W), f32)
    yf = dp("yf", (C, H, W), f32)
    xTp = dp("xTp", (NROW, C), bf16)
    yTp = dp("yTp", (NROW, C), bf16)
    fw = dp("fw", (4, 9, 128, 256), bf16)        # conv lhsT [ci, tap, p_in, m_out]
    dwsc = dp("dwsc", (128, 2, 2, 81), f32)     # [p, ci, img, tap]
    wbun = dp("wbun", (128, _W_TOT), bf16)
    bbun = dp("bbun", (128, _B_TOT), f32)
    ref2 = dp("ref2", (2, 64), f32)

    out_d = nc.declare_dram_parameter("out", [C, HW], f32, isOutput=True)
    hT_d = nc.dram_tensor("hT_scratch", [HW + 1, C], bf16)
    posd = nc.dram_tensor("pos_scratch", [256], f32)

    with tile.TileContext(nc) as tc:
        import contextlib
        with contextlib.ExitStack() as ctx:
            const = ctx.enter_context(tc.tile_pool(name="const", bufs=1))
            work = ctx.enter_context(tc.tile_pool(name="work", bufs=1))

            # ---------- constants ----------
            fw_t = const.tile([128, 36, 256], bf16)
            fw_view = fw[:].rearrange("c t p m -> p (c t) m")
            nc.gpsimd.dma_start(out=fw_t[:, 0:9, :], in_=fw_view[:, 0:9, :])
            dwsc_t = const.tile([128, 2, 2, 81], f32)
            nc.gpsimd.dma_start(out=dwsc_t, in_=dwsc[:])
            wb_t = const.tile([128, _W_TOT], bf16)
            bb_t = const.tile([128, _B_TOT], f32)
            ref_t = const.tile([2, 64], f32)

            pqw_v = wb_t[:, _W_PQW:_W_PQW + 512].rearrange("p (a m) -> p a m", a=4)
            m1w_v = wb_t[:, _W_M1W:_W_M1W + 512].rearrange("p (a m) -> p a m", a=4)
            pkw_v = wb_t[:, _W_PKW:_W_PKW + 512].rearrange("p (a m) -> p a m", a=4)
            pvw_v = wb_t[:, _W_PVW:_W_PVW + 512].rearrange("p (a m) -> p a m", a=4)
            pow_v = wb_t[:, _W_POW:_W_POW + 512].rearrange("p (a m) -> p a m", a=4)
            sw2_v = wb_t[:, _W_SW2:_W_SW2 + 4].rearrange("p (a t) -> p a t", a=2)
            fb_v = bb_t[:, _B_FB:_B_FB + 2]
            pqb_v = bb_t[:, _B_PQB:_B_PQB + 2]
            c1b_v = bb_t[:, _B_C1B:_B_C1B + 2]
            sigb_v = bb_t[:, _B_SIGB:_B_SIGB + 2]
            dwbc_v = bb_t[:, _B_DWBC:_B_DWBC + 4].rearrange("p (a b) -> p a b", a=2)
            lnGc_v = bb_t[:, _B_LNG:_B_LNG + 4].rearrange("p (a b) -> p a b", a=2)
            lnBc_v = bb_t[:, _B_LNB:_B_LNB + 4].rearrange("p (a b) -> p a b", a=2)
            pkb_v = bb_t[:, _B_PKB:_B_PKB + 2]
            pvb_v = bb_t[:, _B_PVB:_B_PVB + 2]
            pob_v = bb_t[:, _B_POB:_B_POB + 2]
            pwx_v = bb_t[:, _B_PWX:_B_PWX + 4].rearrange("p (a t) -> p a t", a=2)
            pwy_v = bb_t[:, _B_PWY:_B_PWY + 4].rearrange("p (a t) -> p a t", a=2)

            ones_r = const.tile([128, 1], f32r)
            nc.vector.memset(ones_r.bitcast(f32), 1.0)
            ones_m = const.tile([128, 32], bf16)
            nc.vector.memset(ones_m, 1.0)
            ident = const.tile([128, 128], bf16)
            make_identity(nc, ident)
            eps_t = const.tile([128, 1], f32)
            nc.vector.memset(eps_t, EPS)
            zrow = const.tile([1, 256], bf16)
            nc.vector.memset(zrow, 0.0)
            nc.sync.dma_start(out=hT_d[HW:HW + 1, :], in_=zrow)

            # persistent activations
            q_t = work.tile([128, 2, HW], bf16)
            k_t = work.tile([128, 2, 128], bf16)
            v_t = work.tile([128, 2, 128], bf16)
            vT8 = work.tile([128, 8, 32], bf16)

            # =======================================================
            # Phase A: conv + offset branch + sampling prep
            # =======================================================
            with tc.tile_pool(name="convin", bufs=1) as cvp, \
                 tc.tile_pool(name="dwp", bufs=2) as dwp, \
                 tc.tile_pool(name="offp", bufs=1) as offp, \
                 tc.tile_pool(name="conv_ps", bufs=2, space="PSUM") as conv_ps, \
                 tc.tile_pool(name="tp_ps", bufs=2, space="PSUM") as tp_ps, \
                 tc.tile_pool(name="sm_ps", bufs=2, space="PSUM") as sm_ps:

                # ----- conv inputs, padded to [72, 72] -----
                pads = []
                pvs = []
                for cidx in range(4):
                    pt = cvp.tile([128, 72 * 72], bf16, name=f"pad{cidx}")
                    pv = pt[:, :].rearrange("p (r c) -> p r c", r=72)
                    nc.vector.memset(pv[:, 0:4, :], 0.0)
                    nc.vector.memset(pv[:, 68:72, :], 0.0)
                    nc.vector.memset(pv[:, 4:68, 0:4], 0.0)
                    nc.vector.memset(pv[:, 4:68, 68:72], 0.0)
                    pads.append(pt)
                    pvs.append(pv)
                fpads = []
                fpvs = []
                for cidx in range(4):
                    ft = cvp.tile([128, 72 * 72], f32, name=f"fpad{cidx}")
                    fv = ft[:, :].rearrange("p (r c) -> p r c", r=72)
                    nc.vector.memset(fv[:, 0:4, :], 0.0)
                    nc.vector.memset(fv[:, 68:72, :], 0.0)
                    nc.vector.memset(fv[:, 4:68, 0:4], 0.0)
                    nc.vector.memset(fv[:, 4:68, 68:72], 0.0)
                    fpads.append(ft)
                    fpvs.append(fv)
                for quarter in range(4):
                    r0, r1 = quarter * 16, quarter * 16 + 16
                    for cidx in range(4):
                        srcq = (xc if cidx < 2 else yc)[(cidx % 2) * 128:(cidx % 2) * 128 + 128]
                        eng = nc.sync if cidx % 2 == 0 else nc.scalar
                        eng.dma_start(out=pvs[cidx][:, 4 + r0:4 + r1, 4:68],
                                      in_=srcq[:, r0:r1, :])
                    if quarter == 0:
                        nc.sync.dma_start(out=fw_t[:, 9:18, :], in_=fw_view[:, 9:18, :])
                        nc.scalar.dma_start(out=fw_t[:, 18:27, :], in_=fw_view[:, 18:27, :])
                        nc.sync.dma_start(out=fw_t[:, 27:36, :], in_=fw_view[:, 27:36, :])
                    if quarter == 2:
                        nc.scalar.dma_start(out=wb_t, in_=wbun[:])
                        nc.scalar.dma_start(out=bb_t, in_=bbun[:])
                        nc.scalar.dma_start(out=ref_t, in_=ref2[:])
                for half in range(2):
                    r0, r1 = half * 32, half * 32 + 32
                    for cidx in range(4):
                        srcq = (xf if cidx < 2 else yf)[(cidx % 2) * 128:(cidx % 2) * 128 + 128]
                        eng = nc.sync if cidx % 2 == 0 else nc.scalar
                        eng.dma_start(out=fpvs[cidx][:, 4 + r0:4 + r1, 4:68],
                                      in_=srcq[:, r0:r1, :])

                # ----- dwconv (DVE) reading bf16 pads, f32 accumulation -----
                hgc = {}
                for img in range(2):
                    accs = []
                    for ci in range(2):
                        pt = fpads[img * 2 + ci]
                        acc576 = dwp.tile([128, 576], f32, tag="a576")
                        tmp576 = dwp.tile([128, 576], f32, tag="t576")
                        for ky in range(9):
                            sl = bass.AP(tensor=pt.tensor, offset=pt.offset + ky * 72,
                                         ap=[pt.ap[0], [576, 8], [8, 8], [1, 9]])
                            wsl = dwsc_t[:, ci, img, ky * 9:(ky + 1) * 9]
                            wbc = bass.AP(tensor=wsl.tensor, offset=wsl.offset,
                                          ap=[wsl.ap[0], [0, 8], [0, 8], [1, 9]])
                            dst = acc576 if ky == 0 else tmp576
                            nc.vector.tensor_tensor(
                                out=dst[:, :].rearrange("p (a b c) -> p a b c", a=8, b=8),
                                in0=sl, in1=wbc, op=ALU.mult)
                            if ky > 0:
                                nc.vector.tensor_tensor(out=acc576, in0=acc576, in1=tmp576, op=ALU.add)
                        acc = offp.tile([128, 64], f32, name=f"dwacc{img}{ci}")
                        rview = bass.AP(tensor=acc576.tensor, offset=acc576.offset,
                                        ap=[acc576.ap[0], [9, 64], [1, 9]])
                        nc.vector.reduce_sum(out=acc, in_=rview, axis=mybir.AxisListType.X)
                        nc.vector.tensor_scalar(out=acc, in0=acc, scalar1=dwbc_v[:, img, ci:ci + 1],
                                                scalar2=None, op0=ALU.add)
                        accs.append(acc)
                    # LN stats over 256 channels (partitions, both chunks) via ones-matmul
                    accr = [offp.tile([128, 64], f32r, name=f"daccr{img}{ci}") for ci in range(2)]
                    sqr = [offp.tile([128, 64], f32r, name=f"dsqr{img}{ci}") for ci in range(2)]
                    for ci in range(2):
                        nc.vector.tensor_copy(accr[ci], accs[ci])
                        nc.vector.tensor_tensor(out=sqr[ci], in0=accs[ci], in1=accs[ci], op=ALU.mult)
                    ps_full = sm_ps.tile([128, 128], f32, tag="sm")
                    ps_st = ps_full[0:1, 0:128]
                    for ci in range(2):
                        nc.tensor.matmul(ps_st[:, 0:64], ones_r, accr[ci], start=(ci == 0), stop=(ci == 1))
                    for ci in range(2):
                        nc.tensor.matmul(ps_st[:, 64:128], ones_r, sqr[ci], start=(ci == 0), stop=(ci == 1))
                    mean1 = offp.tile([1, 64], f32, name=f"m1_{img}")
                    nc.vector.tensor_scalar(out=mean1, in0=ps_st[:, 0:64], scalar1=1.0 / 256.0,
                                            scalar2=None, op0=ALU.mult)
                    ex2 = offp.tile([1, 64], f32, name=f"ex2_{img}")
                    nc.vector.tensor_scalar(out=ex2, in0=ps_st[:, 64:128], scalar1=1.0 / 256.0,
                                            scalar2=None, op0=ALU.mult)
                    msq = offp.tile([1, 64], f32, name=f"msq_{img}")
                    nc.vector.tensor_tensor(out=msq, in0=mean1, in1=mean1, op=ALU.mult)
                    var1 = offp.tile([1, 64], f32, name=f"var_{img}")
                    nc.vector.tensor_tensor(out=var1, in0=ex2, in1=msq, op=ALU.subtract)
                    std1 = offp.tile([1, 64], f32, name=f"std_{img}")
                    nc.scalar.activation(out=std1, in_=var1, func=AF.Sqrt, bias=eps_t[0:1, :], scale=1.0)
                    rstd1 = offp.tile([1, 64], f32, name=f"rstd_{img}")
                    nc.vector.reciprocal(out=rstd1, in_=std1)
                    mbc = offp.tile([128, 64], f32, name=f"mbc_{img}")
                    nc.gpsimd.partition_broadcast(mbc[:], mean1[0:1, :])
                    rbc = offp.tile([128, 64], f32, name=f"rbc_{img}")
                    nc.gpsimd.partition_broadcast(rbc[:], rstd1[0:1, :])
                    hgci = offp.tile([128, 2, 64], f32, name=f"hgc_{img}")
                    for ci in range(2):
                        t2 = dwp.tile([128, 64], f32, tag="dwtmp")
                        nc.vector.tensor_tensor(out=t2, in0=accs[ci], in1=mbc, op=ALU.subtract)
                        nc.vector.tensor_tensor(out=t2, in0=t2, in1=rbc, op=ALU.mult)
                        nc.vector.tensor_scalar(out=t2, in0=t2, scalar1=lnGc_v[:, img, ci:ci + 1],
                                                scalar2=None, op0=ALU.mult)
                        nc.vector.tensor_scalar(out=t2, in0=t2, scalar1=lnBc_v[:, img, ci:ci + 1],
                                                scalar2=None, op0=ALU.add)
                        nc.scalar.activation(out=hgci[:, ci, :], in_=t2, func=AF.Gelu, scale=1.0)
                    hgc[img] = hgci

                pos_sb = offp.tile([2, 2, 64], f32)   # [grid(x,y), (y..x..), 64]
                for g, pw_v in ((0, pwx_v), (1, pwy_v)):
                    pso_full = sm_ps.tile([128, 128], f32, tag="sm")
                    pso = pso_full[0:2, 0:64]
                    for ci in range(2):
                        nc.tensor.matmul(pso, pw_v[:, ci, :], hgc[g][:, ci, :],
                                         start=(ci == 0), stop=(ci == 1))
                    nc.vector.tensor_tensor(out=pos_sb[:, g, :], in0=pso, in1=ref_t, op=ALU.add)
                    nc.vector.tensor_scalar(out=pos_sb[:, g, :], in0=pos_sb[:, g, :],
                                            scalar1=-1.0, scalar2=1.0, op0=ALU.max, op1=ALU.min)
                    # interleave (y, x) pairs into DRAM: posd[g*128 + 2s + t]
                    nc.sync.dma_start(
                        out=bass.AP(tensor=posd, offset=g * 128, ap=[[1, 1], [1, 2], [2, 64]]),
                        in_=pos_sb[:, g, :])
                pos_pt = offp.tile([128, 2], f32)
                nc.sync.dma_start(out=pos_pt, in_=posd.ap().rearrange("(p t) -> p t", t=2))

                # ----- pixel coords, floor, weights, indices (all [128, *]) -----
                pix = offp.tile([128, 2], f32)
                nc.vector.tensor_scalar(out=pix, in0=pos_pt, scalar1=1.0, scalar2=31.5,
                                        op0=ALU.add, op1=ALU.mult)
                ri = offp.tile([128, 2], i32)
                nc.vector.tensor_copy(ri, pix)
                rf = offp.tile([128, 2], f32)
                nc.vector.tensor_copy(rf, ri)
                gt = offp.tile([128, 2], f32)
                nc.vector.tensor_tensor(out=gt, in0=rf, in1=pix, op=ALU.is_gt)
                base = offp.tile([128, 2], f32)
                nc.vector.tensor_tensor(out=base, in0=rf, in1=gt, op=ALU.subtract)
                wf = offp.tile([128, 2], f32)
                nc.vector.tensor_tensor(out=wf, in0=pix, in1=base, op=ALU.subtract)
                y1x1 = offp.tile([128, 2], f32)
                nc.vector.tensor_scalar(out=y1x1, in0=base, scalar1=1.0, scalar2=63.0,
                                        op0=ALU.add, op1=ALU.min)
                omw = offp.tile([128, 2], f32)   # 1 - w
                nc.vector.tensor_scalar(out=omw, in0=wf, scalar1=-1.0, scalar2=1.0,
                                        op0=ALU.mult, op1=ALU.add)
                wq = offp.tile([128, 4], f32)    # w00, w01, w10, w11
                nc.vector.tensor_tensor(out=wq[:, 0:1], in0=omw[:, 1:2], in1=omw[:, 0:1], op=ALU.mult)
                nc.vector.tensor_tensor(out=wq[:, 1:2], in0=wf[:, 1:2], in1=omw[:, 0:1], op=ALU.mult)
                nc.vector.tensor_tensor(out=wq[:, 2:3], in0=omw[:, 1:2], in1=wf[:, 0:1], op=ALU.mult)
                nc.vector.tensor_tensor(out=wq[:, 3:4], in0=wf[:, 1:2], in1=wf[:, 0:1], op=ALU.mult)
                # indices: cols 0=idxP(y0) 1=idxP(y1) 2=idx64(y0) 3=idx64(y1)
                idxf = offp.tile([128, 4], f32)
                nc.vector.tensor_scalar(out=idxf[:, 0:1], in0=base[:, 0:1], scalar1=72.0,
                                        scalar2=292.0, op0=ALU.mult, op1=ALU.add)
                nc.vector.tensor_tensor(out=idxf[:, 0:1], in0=idxf[:, 0:1], in1=base[:, 1:2], op=ALU.add)
                nc.vector.tensor_scalar(out=idxf[:, 1:2], in0=y1x1[:, 0:1], scalar1=72.0,
                                        scalar2=292.0, op0=ALU.mult, op1=ALU.add)
                nc.vector.tensor_tensor(out=idxf[:, 1:2], in0=idxf[:, 1:2], in1=base[:, 1:2], op=ALU.add)
                nc.vector.tensor_scalar(out=idxf[:, 2:3], in0=base[:, 0:1], scalar1=64.0,
                                        scalar2=None, op0=ALU.mult)
                nc.vector.tensor_tensor(out=idxf[:, 2:3], in0=idxf[:, 2:3], in1=base[:, 1:2], op=ALU.add)
                nc.vector.tensor_scalar(out=idxf[:, 3:4], in0=y1x1[:, 0:1], scalar1=64.0,
                                        scalar2=None, op0=ALU.mult)
                nc.vector.tensor_tensor(out=idxf[:, 3:4], in0=idxf[:, 3:4], in1=base[:, 1:2], op=ALU.add)
                idxi = offp.tile([128, 4], i32)
                nc.vector.tensor_copy(idxi, idxf)

                # ----- strip gathers + bilinear (x/y early: hidden under conv) -----
                def strip_gather(table, col):
                    g = dwp.tile([128, 512], bf16, tag="strip")
                    nc.gpsimd.indirect_dma_start(
                        out=g[:], out_offset=None, in_=table,
                        in_offset=bass.IndirectOffsetOnAxis(ap=idxi[:, col:col + 1], axis=0))
                    return g

                def bilin(g0, g1, name):
                    o = offp.tile([128, 256], bf16, name=name)
                    tmp = offp.tile([128, 256], bf16, tag="btmp")
                    nc.vector.tensor_scalar(out=o, in0=g0[:, 0:256], scalar1=wq[:, 0:1], scalar2=None, op0=ALU.mult)
                    nc.vector.tensor_scalar(out=tmp, in0=g0[:, 256:512], scalar1=wq[:, 1:2], scalar2=None, op0=ALU.mult)
                    nc.vector.tensor_tensor(out=o, in0=o, in1=tmp, op=ALU.add)
                    nc.vector.tensor_scalar(out=tmp, in0=g1[:, 0:256], scalar1=wq[:, 2:3], scalar2=None, op0=ALU.mult)
                    nc.vector.tensor_tensor(out=o, in0=o, in1=tmp, op=ALU.add)
                    nc.vector.tensor_scalar(out=tmp, in0=g1[:, 256:512], scalar1=wq[:, 3:4], scalar2=None, op0=ALU.mult)
                    nc.vector.tensor_tensor(out=o, in0=o, in1=tmp, op=ALU.add)
                    return o

                xsT = bilin(strip_gather(xTp[:], 0), strip_gather(xTp[:], 1), "xsT")
                ysT = bilin(strip_gather(yTp[:], 0), strip_gather(yTp[:], 1), "ysT")

                # ----- conv3x3 matmuls + gelu + fused projq + hT transposes -----
                for rb in range(8):
                    hb = dwp.tile([128, 2, 512], bf16, tag="hblk")
                    for mo in range(2):
                        ps = conv_ps.tile([128, 512], f32, tag="cps")
                        first = True
                        for ci in range(4):
                            pv = pvs[ci]
                            for tap in range(9):
                                ky, kx = tap // 3, tap % 3
                                rhs = pv[:, rb * 8 + ky + 3: rb * 8 + ky + 11, kx + 3:kx + 67]
                                nc.tensor.matmul(ps, fw_t[:, ci * 9 + tap, ts(mo, 128)], rhs,
                                                 start=first, stop=(ci == 3 and tap == 8))
                                first = False
                        nc.scalar.activation(out=hb[:, mo, :], in_=ps,
                                             func=AF.Gelu, bias=fb_v[:, mo:mo + 1], scale=1.0)
                    for mo in range(2):
                        ps = conv_ps.tile([128, 512], f32, tag="cps")
                        for ci in range(2):
                            nc.tensor.matmul(ps, pqw_v[:, ci * 2 + mo, :], hb[:, ci, :],
                                             start=(ci == 0), stop=(ci == 1))
                        nc.scalar.activation(out=q_t[:, mo, ts(rb, 512)], in_=ps,
                                             func=AF.Identity, bias=pqb_v[:, mo:mo + 1], scale=1.0)
                    for ci in range(2):
                        tpq = tp_ps.tile([128, 4, 128], bf16, tag="tp4")
                        for s4 in range(4):
                            nc.tensor.matmul(tpq[:, s4, :], hb[:, ci, ts(s4, 128)], ident,
                                             is_transpose=True, start=True, stop=True)
                        stg = dwp.tile([128, 4, 128], bf16, tag="hstage")
                        nc.scalar.activation(out=stg, in_=tpq, func=AF.Copy, bias=0.0, scale=1.0)
                        nc.sync.dma_start(
                            out=bass.AP(tensor=hT_d, offset=(rb * 512) * 256 + ci * 128,
                                        ap=[[256, 128], [128 * 256, 4], [1, 128]]),
                            in_=stg)

                # ----- sampled h + sw branch -----
                hsT = bilin(strip_gather(hT_d[:], 2), strip_gather(hT_d[:], 3), "hsT")
                hs = offp.tile([128, 2, 128], bf16)
                for ci in range(2):
                    tp_full = tp_ps.tile([128, 4, 128], bf16, tag="tp4")
                    tp = tp_full[:, 0, :]
                    nc.tensor.matmul(tp, hsT[:, ts(ci, 128)], ident,
                                     is_transpose=True, start=True, stop=True)
                    nc.vector.tensor_copy(hs[:, ci, :], tp)
                zr = offp.tile([128, 2, 128], bf16)
                for mo in range(2):
                    ps = sm_ps.tile([128, 128], f32, tag="sm")
                    for ci in range(2):
                        nc.tensor.matmul(ps, m1w_v[:, ci * 2 + mo, :], hs[:, ci, :],
                                         start=(ci == 0), stop=(ci == 1))
                    nc.scalar.activation(out=zr[:, mo, :], in_=ps, func=AF.Relu,
                                         bias=c1b_v[:, mo:mo + 1], scale=1.0)
                # S^T = Z^T @ sw2^T directly: [128 samples, 2]
                psT_full = sm_ps.tile([128, 128], f32, tag="sm")
                psT = psT_full[:, 0:2]
                for ci in range(2):
                    nc.tensor.matmul(psT, zr[:, ci, :], sw2_v[:, ci, :],
                                     start=(ci == 0), stop=(ci == 1))
                ST = offp.tile([128, 2], f32)
                nc.vector.tensor_copy(ST, psT)
                dS = offp.tile([128, 1], f32)
                nc.vector.tensor_tensor(out=dS, in0=ST[:, 0:1], in1=ST[:, 1:2], op=ALU.subtract)
                sw0 = offp.tile([128, 1], f32)
                nc.scalar.activation(out=sw0, in_=dS, func=AF.Sigmoid, bias=sigb_v[:, 0:1], scale=1.0)
                sw1 = offp.tile([128, 1], f32)
                nc.scalar.activation(out=sw1, in_=dS, func=AF.Sigmoid, bias=sigb_v[:, 1:2], scale=-1.0)

                # ----- sampled mix + transpose; k/v proj; vT8 -----
                smT = offp.tile([128, 256], bf16)
                tmp2 = offp.tile([128, 256], bf16)
                nc.vector.tensor_scalar(out=smT, in0=xsT, scalar1=sw0, scalar2=None, op0=ALU.mult)
                nc.vector.tensor_scalar(out=tmp2, in0=ysT, scalar1=sw1, scalar2=None, op0=ALU.mult)
                nc.vector.tensor_tensor(out=smT, in0=smT, in1=tmp2, op=ALU.add)
                smpl = offp.tile([128, 2, 128], bf16)
                for ci in range(2):
                    tp_full = tp_ps.tile([128, 4, 128], bf16, tag="tp4")
                    tp = tp_full[:, 0, :]
                    nc.tensor.matmul(tp, smT[:, ts(ci, 128)], ident,
                                     is_transpose=True, start=True, stop=True)
                    nc.vector.tensor_copy(smpl[:, ci, :], tp)
                for dst, wv, bv in ((k_t, pkw_v, pkb_v), (v_t, pvw_v, pvb_v)):
                    for mo in range(2):
                        ps = sm_ps.tile([128, 128], f32, tag="sm")
                        for ci in range(2):
                            nc.tensor.matmul(ps, wv[:, ci * 2 + mo, :], smpl[:, ci, :],
                                             start=(ci == 0), stop=(ci == 1))
                        nc.scalar.activation(out=dst[:, mo, :], in_=ps, func=AF.Identity,
                                             bias=bv[:, mo:mo + 1], scale=1.0)
                for ci in range(2):
                    tp_full = tp_ps.tile([128, 4, 128], bf16, tag="tp4")
                    tp = tp_full[:, 0, :]
                    nc.tensor.matmul(tp, v_t[:, ci, :], ident,
                                     is_transpose=True, start=True, stop=True)
                    nc.vector.tensor_copy(vT8[:, ci * 4:(ci + 1) * 4, :],
                                          tp[:, :].rearrange("p (a b) -> p a b", a=4))

            # =======================================================
            # Phase B: attention + output projection
            # =======================================================
            with tc.tile_pool(name="apool", bufs=1) as apool, \
                 tc.tile_pool(name="epool", bufs=3) as epool, \
                 tc.tile_pool(name="npool", bufs=4) as npool, \
                 tc.tile_pool(name="opool", bufs=3) as opool, \
                 tc.tile_pool(name="qk_ps", bufs=2, space="PSUM") as qk_ps, \
                 tc.tile_pool(name="av_ps", bufs=2, space="PSUM") as av_ps, \
                 tc.tile_pool(name="po_ps", bufs=2, space="PSUM") as po_ps:

                att_t = apool.tile([128, 2, HW], bf16)
                E_tiles = {}

                def stage_qk(nb):
                    E = epool.tile([128, 8, 512], bf16, tag="E")
                    for hg4 in range(2):
                        qks = []
                        for j in range(4):
                            qk = qk_ps.tile([128, 512], f32, tag="qk")
                            nc.tensor.matmul(qk, k_t[ts(j, 32), hg4, :],
                                             q_t[ts(j, 32), hg4, ts(nb, 512)],
                                             start=True, stop=True,
                                             tile_position=(32 * j, 0))
                            qks.append(qk)
                        for j in range(4):
                            nc.scalar.activation(out=E[:, hg4 * 4 + j, :], in_=qks[j],
                                                 func=AF.Exp, scale=SCALE)
                    E_tiles[nb] = E

                def stage_av(nb):
                    E = E_tiles.pop(nb)
                    for g in range(2):
                        avg = av_ps.tile([128, 512], f32, tag="avg")
                        ps_s = av_ps.tile([128, 512], f32, tag="sums")
                        for j in range(4):
                            hh = g * 4 + j
                            nc.tensor.matmul(avg[ts(j, 32), :], vT8[:, hh, :], E[:, hh, :],
                                             start=True, stop=True, tile_position=(0, 32 * j))
                            nc.tensor.matmul(ps_s[ts(j, 32), :], ones_m, E[:, hh, :],
                                             start=True, stop=True, tile_position=(0, 32 * j))
                        rec = npool.tile([128, 512], f32, tag="rec")
                        nc.vector.reciprocal(out=rec, in_=ps_s)
                        nc.vector.tensor_tensor(out=att_t[:, g, ts(nb, 512)],
                                                in0=avg, in1=rec, op=ALU.mult)

                def stage_po(nb):
                    ot = opool.tile([128, 2, 512], f32, tag="ot")
                    for mo in range(2):
                        ps = po_ps.tile([128, 512], f32, tag="po")
                        for ci in range(2):
                            nc.tensor.matmul(ps, pow_v[:, ci * 2 + mo, :], att_t[:, ci, ts(nb, 512)],
                                             start=(ci == 0), stop=(ci == 1))
                        nc.scalar.activation(out=ot[:, mo, :], in_=ps, func=AF.Identity,
                                             bias=pob_v[:, mo:mo + 1], scale=1.0)
                    nc.scalar.dma_start(
                        out=bass.AP(tensor=out_d, offset=nb * 512,
                                    ap=[[HW, 128], [128 * HW, 2], [1, 512]]),
                        in_=ot)

                for step in range(10):
                    if step < 8:
                        stage_qk(step)
                    if 1 <= step <= 8:
                        stage_av(step - 1)
                    if step >= 2:
                        stage_po(step - 2)

    nc.finalize()
    return nc


def _host_prep(inp):
    g = {k: np.ascontiguousarray(np.asarray(v, dtype=np.float32)) for k, v in inp.items()}
    s = g['bn_g'] / np.sqrt(g['bn_v'] + EPS)
    fwf = g['fuse_w'] * s[:, None, None, None]          # [256, 512, 3, 3]
    fbf = (g['fuse_b'] - g['bn_m']) * s + g['bn_b']
    M1 = g['sw1_w'] @ g['projq_w']
    c1 = g['sw1_w'] @ g['projq_b'] + g['sw1_b']
    bf = ml_dtypes.bfloat16

    def lhsT4(wmat):  # [out, in] -> [128, 4*128] with a = ci*2+mo
        a = np.zeros((128, 4, 128), np.float32)
        for ci in range(2):
            for mo in range(2):
                a[:, ci * 2 + mo, :] = wmat[mo * 128:(mo + 1) * 128, ci * 128:(ci + 1) * 128].T
        return a.reshape(128, 512)

    def b2(vec):  # [256] -> [128, 2]
        return np.stack([vec[0:128], vec[128:256]], 1).astype(np.float32)

    d = {}
    fw_a = np.zeros((4, 9, 128, 256), np.float32)
    for ci in range(4):
        for ky in range(3):
            for kx in range(3):
                fw_a[ci, ky * 3 + kx] = fwf[:, ci * 128:(ci + 1) * 128, ky, kx].T
    d['fw'] = fw_a.astype(bf)

    sw2a = np.zeros((128, 2, 2), np.float32)   # [p, ci, t]
    for ci in range(2):
        sw2a[:, ci, :] = g['sw2_w'][:, ci * 128:(ci + 1) * 128].T
    wbun = np.concatenate([lhsT4(g['projq_w']), lhsT4(M1), lhsT4(g['projk_w']),
                           lhsT4(g['projv_w']), lhsT4(g['projo_w']),
                           sw2a.reshape(128, 4)], axis=1)
    assert wbun.shape[1] == _W_TOT
    d['wbun'] = wbun.astype(bf)

    db = float(g['sw2_b'][0] - g['sw2_b'][1])
    sigb = np.tile(np.array([[db, -db]], np.float32), (128, 1))
    dwbc = np.zeros((128, 2, 2), np.float32)   # [p, img, ci]
    lnGc = np.zeros((128, 2, 2), np.float32)
    lnBc = np.zeros((128, 2, 2), np.float32)
    dwsc = np.zeros((128, 2, 2, 81), np.float32)
    for img, pre in ((0, 'offx'), (1, 'offy')):
        w = g[pre + '_dw_w'][:, 0].reshape(256, 81)
        for ci in range(2):
            dwsc[:, ci, img, :] = w[ci * 128:(ci + 1) * 128]
            dwbc[:, img, ci] = g[pre + '_dw_b'][ci * 128:(ci + 1) * 128]
            lnGc[:, img, ci] = g[pre + '_ln_g'][ci * 128:(ci + 1) * 128]
            lnBc[:, img, ci] = g[pre + '_ln_b'][ci * 128:(ci + 1) * 128]
    d['dwsc'] = np.ascontiguousarray(dwsc, np.float32)
    pwx = np.zeros((128, 2, 2), np.float32)    # [p, ci, t]
    pwy = np.zeros((128, 2, 2), np.float32)
    for ci in range(2):
        pwx[:, ci, :] = g['offx_pw_w'][:, ci * 128:(ci + 1) * 128].T
        pwy[:, ci, :] = g['offy_pw_w'][:, ci * 128:(ci + 1) * 128].T
    bbun = np.concatenate([b2(fbf), b2(g['projq_b']), b2(c1), sigb,
                           dwbc.reshape(128, 4), lnGc.reshape(128, 4), lnBc.reshape(128, 4),
                           b2(g['projk_b']), b2(g['projv_b']), b2(g['projo_b']),
                           pwx.reshape(128, 4), pwy.reshape(128, 4)], axis=1)
    assert bbun.shape[1] == _B_TOT
    d['bbun'] = np.ascontiguousarray(bbun, np.float32)

    ry = (np.linspace(0.5, Hk - 0.5, Hk, dtype=np.float32) / np.float32(Hk - 1.0)) * 2.0 - 1.0
    gy, gx = np.meshgrid(ry, ry, indexing='ij')
    d['ref2'] = np.stack([gy, gx], 0).reshape(2, 64).astype(np.float32)
    return g, d


def kernel(**inputs):
    from concourse.bass_utils import run_bass_kernel_spmd

    if 'nc' not in _CACHE:
        _CACHE['nc'] = _build_program()
    nc = _CACHE['nc']

    g, wd = _host_prep(inputs)
    bf = ml_dtypes.bfloat16
    in_maps = []
    for b in range(B):
        m = dict(wd)
        xb = g['x'][b]
        yb = g['y'][b]
        m['xc'] = xb.astype(bf)
        m['yc'] = yb.astype(bf)
        m['xf'] = xb
        m['yf'] = yb
        for nm, img in (('xTp', xb), ('yTp', yb)):
            t = np.zeros((PADR, PADR, C), np.float32)
            t[4:68, 4:68] = img.transpose(1, 2, 0)
            m[nm] = t.reshape(NROW, C).astype(bf)
        in_maps.append(m)

    res = run_bass_kernel_spmd(nc, in_maps, list(range(B)))
    out = np.stack([res.results[i]['out'].reshape(C, H, W) for i in range(B)])
    return out.astype(np.float32)
